# revision 1
# baseline (speedup 1.0000x reference)
"""Expert-parallel CMoE kernel for 8 Trainium2 NeuronCores (v5).

Sharding (hardcoded for B=8, T=2048, D=1024, F=2048, E=16, C=1024):
  core k owns batch k (token shift, receptance, output) and experts
  {2k, 2k+1} (FFN). Hash routing is int math on token_ids, done on host;
  the resulting permutations ship to the cores as index tensors.

Key scheduling facts this version is built around (from HW traces):
  - Scatters into one tile serialize on each other's DMA completion
    (Tile WAW) -> use ONE dma_scatter_add per chunk into zero-filled
    buffers (pad rows are never gathered, so only written rows need 0).
  - HWDGE xbar dma_start_transpose serializes globally against
    collectives and other DMA modes -> never use it; the xr transpose
    for receptance uses the SWDGE transposing dma_gather (iota idx)
    like the dispatch-side XT gathers.
  - Big partition-contiguous "(p a)" loads keep descriptor counts low.

Pipeline: phase A runs 4 chunks of 512 tokens (2 loads, 5 DVE ops, one
scatter, quarter A2A, then receptance for the chunk: xr store -> gather
-transpose -> PE matmuls -> sigmoid).  FFN per expert with early
single-buffered weight loads; combine is 4 A2As (expert x half).
Phase D gathers y, multiplies by r, stores fp32.
"""
import sys

for _p in ("/opt/trn_rl_repo", "/root/.axon_site/_ro/trn_rl_repo"):
    if _p not in sys.path:
        sys.path.append(_p)

import numpy as np
import ml_dtypes

import concourse.bass as bass
import concourse.bacc as bacc
import concourse.mybir as mybir
import concourse.tile as tile
from concourse.tile import add_dep_helper
from concourse.bass_utils import run_bass_kernel_spmd

P = 128
B, T, D, F, E = 8, 2048, 1024, 2048, 16
N = B * T
C = max(4, N // E)          # 1024
HASH_PRIME = 5099
NCORES = 8
EPC = E // NCORES           # experts per core = 2
NQ = 4                      # dispatch quarters
QT = T // NQ                # 512 tokens per dispatch chunk
NCK = 2                     # combine halves per expert
CH = C // NCK               # 512 slots per combine chunk
BF16 = mybir.dt.bfloat16
F32 = mybir.dt.float32
I16 = mybir.dt.int16
I32 = mybir.dt.int32
nbf16 = ml_dtypes.bfloat16
AF = mybir.ActivationFunctionType

_CACHE = {}


def _r16(v):
    return int(-(-int(v) // 16) * 16)


def _wrap16(a):
    a = np.asarray(a, np.int16)
    w = a.reshape(-1, 16).T.copy()       # j at [j%16, j//16]
    return np.tile(w, (8, 1))            # replicated across 8 Q7 cores


def _route(token_ids):
    tid = np.asarray(token_ids).reshape(N).astype(np.int64)
    e = (tid * HASH_PRIME) % E
    onehot = (e[:, None] == np.arange(E)).astype(np.int64)
    pos = onehot.cumsum(0)[np.arange(N), e] - 1
    keep = pos < C
    return e, pos, keep


def _build_indices(token_ids):
    e, pos, keep = _route(token_ids)
    src = np.arange(N) // T
    dst = e // EPC
    el = e % EPC
    local_t = np.arange(N) % T

    def pack(mask):
        rank = np.zeros(N, np.int64)
        cnt = np.zeros((NCORES, NCORES), np.int64)
        for n in np.nonzero(mask)[0]:
            rank[n] = cnt[src[n], dst[n]]
            cnt[src[n], dst[n]] += 1
        return rank, _r16(max(cnt.max(), 1))

    # ---- dispatch: 2 chunks by destination expert parity, so the
    # expert-0 FFN can start after a single A2A
    de = [pack(keep & (el == e)) for e in range(EPC)]
    Kq = tuple(k for _, k in de)
    OFF1 = np.concatenate([[0], np.cumsum([NCORES * k for k in Kq])])
    R1 = int(OFF1[-1])                   # trash row in recv1

    recv_row = np.full((NCORES, EPC * C), R1, np.int64)
    for e in range(EPC):
        rank, K = de[e]
        for n in np.nonzero(keep & (el == e))[0]:
            recv_row[dst[n], el[n] * C + pos[n]] = \
                OFF1[e] + src[n] * K + rank[n]

    # ---- combine: 4 chunks by (expert parity, capacity half)
    order = [(eli, ck) for eli in range(EPC) for ck in range(NCK)]
    comb = {c: pack(keep & (el == c[0]) & (pos // CH == c[1]))
            for c in order}
    K2 = tuple(comb[c][1] for c in order)
    OFF2 = {}
    acc = 0
    for c, k in zip(order, K2):
        OFF2[c] = acc
        acc += NCORES * k
    R2 = acc                             # trash row in recv2

    sl2 = np.zeros((NCORES, EPC, C), np.int64)
    for c, k in zip(order, K2):
        sl2[:, c[0], c[1] * CH:(c[1] + 1) * CH] = NCORES * k
    ygather = np.full(N, R2, np.int64)
    for n in np.nonzero(keep)[0]:
        c = (el[n], pos[n] // CH)
        rank, k = comb[c]
        sl2[dst[n], el[n], pos[n]] = src[n] * k + rank[n]
        ygather[n] = OFF2[c] + dst[n] * k + rank[n]

    # dispatch scatter position j <-> xk_all[p=j%128, c=j//128]
    # = token (c//4)*512 + 4p + (c%4)
    jj = np.arange(T)
    tok_of_j = (jj // P // 4) * 512 + 4 * (jj % P) + (jj // P) % 4

    per_core = []
    for k in range(NCORES):
        tok = slice(k * T, (k + 1) * T)
        sds = []
        for e in range(EPC):
            rank, K = de[e]
            idx_e = np.where(keep & (el == e), dst * K + rank, NCORES * K)
            sds.append(_wrap16(idx_e[tok][tok_of_j]))
        sd = np.concatenate(sds, axis=1)
        # combine scatter idx: position j = tt*128+p <-> slot ck*512+j
        sc = np.concatenate(
            [_wrap16(sl2[k, c[0], c[1] * CH:(c[1] + 1) * CH])
             for c in order], axis=1)
        per_core.append({
            "sd16": sd,
            "slot16": _wrap16(recv_row[k]),
            "sc16": sc,
            "ygather16": _wrap16(ygather[tok]),
        })
    return (Kq, K2), per_core


def _build_nc(cfg):
    Kq, K2 = cfg
    OFF1 = np.concatenate([[0], np.cumsum([NCORES * k for k in Kq])])
    R1 = int(OFF1[-1])
    order = [(eli, ck) for eli in range(EPC) for ck in range(NCK)]
    OFF2 = {}
    acc = 0
    for c, k in zip(order, K2):
        OFF2[c] = acc
        acc += NCORES * k
    R2 = acc
    K2d = dict(zip(order, K2))

    nc = bacc.Bacc("TRN2", target_bir_lowering=False, debug=False,
                   num_devices=NCORES)

    x_ext = nc.dram_tensor("x_ext", [T + 1, D], BF16, kind="ExternalInput")
    maa_k = nc.dram_tensor("maa_k", [1, D], BF16, kind="ExternalInput")
    maa_r = nc.dram_tensor("maa_r", [1, D], BF16, kind="ExternalInput")
    wrt = nc.dram_tensor("wrt", [D, D], BF16, kind="ExternalInput")
    wk = nc.dram_tensor("wk", [EPC, D, F], BF16, kind="ExternalInput")
    wv = nc.dram_tensor("wv", [EPC, F, D], BF16, kind="ExternalInput")
    sd16 = nc.dram_tensor("sd16", [P, EPC * T // 16], I16,
                          kind="ExternalInput")
    slot16 = nc.dram_tensor("slot16", [P, EPC * C // 16], I16,
                            kind="ExternalInput")
    sc16 = nc.dram_tensor("sc16", [P, EPC * C // 16], I16,
                          kind="ExternalInput")
    ygather16 = nc.dram_tensor("ygather16", [P, T // 16], I16,
                               kind="ExternalInput")
    iota16 = nc.dram_tensor("iota16", [P, QT // 16], I16,
                            kind="ExternalInput")
    out = nc.dram_tensor("out", [T, D], F32, kind="ExternalOutput")

    DC = D // P          # 8
    FC = F // P          # 16
    rg = [list(range(NCORES))]

    with tile.TileContext(nc) as tc:
        with (
            tc.tile_pool(name="dram", bufs=1, space="DRAM") as dram,
            tc.tile_pool(name="misc", bufs=1) as misc,
            tc.tile_pool(name="pwk", bufs=1) as pwk,
            tc.tile_pool(name="pwv", bufs=1) as pwv,
            tc.tile_pool(name="psh", bufs=2, space="PSUM") as psh,
            tc.tile_pool(name="psy", bufs=2, space="PSUM") as psy,
        ):
            disp = [dram.tile([NCORES * Kq[e] + 1, D], BF16, name=f"disp{e}")
                    for e in range(EPC)]
            recv1 = dram.tile([R1 + 1, D], BF16)
            a2 = {c: dram.tile([NCORES * K2d[c] + 1, D], BF16,
                               name=f"a2_{c[0]}_{c[1]}")
                  for c in order}
            recv2 = dram.tile([R2 + 1, D], BF16)
            r_buf = dram.tile([T, D], BF16)

            zrow = misc.tile([1, D], BF16)
            nc.vector.memzero(zrow[:])
            nc.scalar.dma_start(out=recv1[R1:R1 + 1, :], in_=zrow[:])
            nc.scalar.dma_start(out=recv2[R2:R2 + 1, :], in_=zrow[:])

            # dispatch index on sync (needed first), the rest on scalar
            sD = misc.tile([P, EPC * T // 16], I16)
            nc.sync.dma_start(out=sD[:], in_=sd16[:])
            sl16 = misc.tile([P, EPC * C // 16], I16)
            nc.scalar.dma_start(out=sl16[:], in_=slot16[:])
            sC = misc.tile([P, EPC * C // 16], I16)
            nc.scalar.dma_start(out=sC[:], in_=sc16[:])
            yg16 = misc.tile([P, T // 16], I16)
            nc.scalar.dma_start(out=yg16[:], in_=ygather16[:])
            io16 = misc.tile([P, QT // 16], I16)
            nc.scalar.dma_start(out=io16[:], in_=iota16[:])

            # zero-fill the scatter-add target regions (pad rows are never
            # gathered on the recv side, but written rows need 0 for +=)
            ZR = 4
            zb = misc.tile([P, ZR, D], BF16)
            nc.vector.memzero(zb[:])

            def zero_fill(buf, rows):
                for off in range(0, rows, ZR * P):
                    n = min(ZR * P, rows - off)
                    nc.scalar.dma_start(
                        out=buf[off:off + n, :].rearrange(
                            "(a p) d -> p a d", p=P),
                        in_=zb[:, 0:n // P, :])


            wk_t = [pwk.tile([P, DC, F], BF16, tag="wk", name=f"wk_t{i}")
                    for i in range(EPC)]
            wv_t = [pwv.tile([P, FC, D], BF16, tag="wv", name=f"wv_t{i}")
                    for i in range(EPC)]

            # ---- phase A (token shift) + receptance, 4 chunks of 512
            with (
                tc.tile_pool(name="pa", bufs=2) as pa,
                tc.tile_pool(name="pxk", bufs=1) as pxk,
                tc.tile_pool(name="pdx", bufs=3) as pdx,
                tc.tile_pool(name="pam", bufs=1) as pam,
                tc.tile_pool(name="prx", bufs=2) as prx,
                tc.tile_pool(name="prs", bufs=1) as prs,
                tc.tile_pool(name="psr", bufs=1, space="PSUM") as psr,
            ):
                maakb = pam.tile([P, D], BF16)
                maarb = pam.tile([P, D], BF16)
                nc.scalar.dma_start(out=maakb[:],
                                    in_=maa_k[:].to_broadcast([P, D]))
                nc.scalar.dma_start(out=maarb[:],
                                    in_=maa_r[:].to_broadcast([P, D]))
                # wrt ships pre-shuffled so this is partition-contiguous
                wrt_sb = pam.tile([P, DC, D], BF16)
                nc.scalar.dma_start(out=wrt_sb[:],
                                    in_=wrt.rearrange("(p c) e -> p c e",
                                                      p=P))
                # zero-fills go on the scalar queue AFTER maa/wrt (those
                # gate the first DVE ops / receptance matmuls)
                for e in range(EPC):
                    zero_fill(disp[e], NCORES * Kq[e])

                # xk accumulates across all 4 chunks; the two expert-parity
                # scatters read the whole thing after the loop
                xk_all = pxk.tile([P, NQ * 4, D], BF16)
                for q in range(NQ):
                    xq = pa.tile([P, 4, D], BF16, tag="xq")
                    nc.sync.dma_start(
                        out=xq[:],
                        in_=x_ext[1 + q * QT:1 + (q + 1) * QT, :].rearrange(
                            "(p a) d -> p a d", p=P))
                    # xprev strip for a=0: tokens 4p-1 = x_ext rows q*QT+4p
                    xp0 = pa.tile([P, 1, D], BF16, tag="xp0")
                    nc.sync.dma_start(
                        out=xp0[:],
                        in_=x_ext[q * QT:(q + 1) * QT, :].rearrange(
                            "(p a) d -> p a d", p=P)[:, 0:1, :])
                    # xprev for a=1..3 is xq shifted by one within the tile
                    dx = pdx.tile([P, 4, D], BF16, tag="dx")
                    nc.vector.tensor_sub(out=dx[:, 0:1, :], in0=xp0[:],
                                         in1=xq[:, 0:1, :])
                    nc.vector.tensor_sub(out=dx[:, 1:4, :],
                                         in0=xq[:, 0:3, :], in1=xq[:, 1:4, :])
                    xks = xk_all[:, 4 * q:4 * q + 4, :]
                    for n in range(4):
                        nc.vector.tensor_mul(out=xk_all[:, 4 * q + n, :],
                                             in0=dx[:, n, :], in1=maakb[:])
                    nc.vector.tensor_add(out=xks, in0=xks, in1=xq[:])

                    # xr built in-place in dx (dx is dead after this)
                    for n in range(4):
                        nc.vector.tensor_mul(out=dx[:, n, :],
                                             in0=dx[:, n, :], in1=maarb[:])
                    nc.vector.tensor_add(out=dx[:], in0=dx[:], in1=xq[:])

                    # receptance for this chunk (PE soaks while A2A flies):
                    # SBUF-source transposing gather straight from the xr
                    # tile -- no DRAM round trip. Layout maps via
                    # tokens_per_rank=128: idx value = a*128+p.
                    xrT = prx.tile([P, DC, QT], BF16, tag="xrT")
                    nc.gpsimd.dma_gather(
                        out_ap=xrT[:], in_ap=dx[:],
                        idxs_ap=io16[:],
                        num_idxs=QT, num_idxs_reg=QT, elem_size=D,
                        transpose=True,
                        sbuf_tokens_per_rank=P,
                        sbuf_free_dim_per_rank=D * 2)
                    if q == NQ - 1:
                        # both expert-parity scatters, then both triggers
                        for e in range(EPC):
                            nc.gpsimd.dma_scatter_add(
                                out_ap=disp[e][:], in_ap=xk_all[:],
                                idxs_ap=sD[:, e * 128:(e + 1) * 128],
                                num_idxs=T, num_idxs_reg=T, elem_size=D)
                        for e in range(EPC):
                            last_trig = nc.gpsimd.collective_compute(
                                "AllToAll", mybir.AluOpType.bypass,
                                replica_groups=rg,
                                ins=[disp[e][0:NCORES * Kq[e], :]],
                                outs=[recv1[int(OFF1[e]):
                                            int(OFF1[e + 1]), :]])
                    rsb = prs.tile([P, 4, D], BF16, tag="rsb")
                    for tt in range(4):
                        pr0 = psr.tile([P, 512], F32, space="PSUM", tag="pr0")
                        pr1 = psr.tile([P, 512], F32, space="PSUM", tag="pr1")
                        for dc in range(DC):
                            nc.tensor.matmul(
                                out=pr0[:],
                                lhsT=xrT[:, dc, tt * P:(tt + 1) * P],
                                rhs=wrt_sb[:, dc, 0:512],
                                start=(dc == 0), stop=(dc == DC - 1))
                            nc.tensor.matmul(
                                out=pr1[:],
                                lhsT=xrT[:, dc, tt * P:(tt + 1) * P],
                                rhs=wrt_sb[:, dc, 512:1024],
                                start=(dc == 0), stop=(dc == DC - 1))
                        nc.scalar.activation(out=rsb[:, tt, 0:512],
                                             in_=pr0[:], func=AF.Sigmoid)
                        nc.scalar.activation(out=rsb[:, tt, 512:1024],
                                             in_=pr1[:], func=AF.Sigmoid)
                    nc.scalar.dma_start(
                        out=r_buf[q * QT:(q + 1) * QT, :].rearrange(
                            "(a p) d -> p a d", p=P),
                        in_=rsb[:])

            # expert-0 weight loads: held back behind the last dispatch
            # trigger so they don't steal HBM from the phase-A window
            wl0 = nc.sync.dma_start(
                out=wk_t[0][:], in_=wk[0].rearrange("(p c) f -> p c f", p=P))
            add_dep_helper(wl0.ins, last_trig.ins,
                           reason="keep wk0 load out of the phase-A window")
            nc.sync.dma_start(out=wv_t[0][:],
                              in_=wv[0].rearrange("(p c) f -> p c f", p=P))

            # zero-fill combine scatter targets (first use is mid-FFN)
            for c in order:
                zero_fill(a2[c], NCORES * K2d[c])

            # ---------------- phase C: expert FFNs
            with (
                tc.tile_pool(name="pfx", bufs=2) as pfx,
                tc.tile_pool(name="pfh", bufs=1) as pfh,
                tc.tile_pool(name="pfr", bufs=2) as pfr,
                tc.tile_pool(name="pfy", bufs=2) as pfy,
            ):
                for elp in range(EPC):
                    if elp > 0:
                        nc.sync.dma_start(
                            out=wk_t[elp][:],
                            in_=wk[elp].rearrange("(p c) f -> p c f", p=P))
                        nc.sync.dma_start(
                            out=wv_t[elp][:],
                            in_=wv[elp].rearrange("(p c) f -> p c f", p=P))
                    wk_sb, wv_sb = wk_t[elp], wv_t[elp]
                    for ck in range(NCK):
                        XT = pfx.tile([P, DC, 512], BF16, tag="XT")
                        col0 = (elp * C + ck * CH) // 16
                        nc.gpsimd.dma_gather(
                            out_ap=XT[:], in_ap=recv1[:],
                            idxs_ap=sl16[:, col0:col0 + 32],
                            num_idxs=512, num_idxs_reg=512, elem_size=D,
                            transpose=True)
                        ht = pfh.tile([P, FC, 512], BF16, tag="ht")
                        for ft in range(FC):
                            ph = psh.tile([P, 512], F32, space="PSUM",
                                          tag="ph")
                            for dc in range(DC):
                                nc.tensor.matmul(
                                    out=ph[:],
                                    lhsT=wk_sb[:, dc, ft * P:(ft + 1) * P],
                                    rhs=XT[:, dc, :],
                                    start=(dc == 0), stop=(dc == DC - 1))
                            hr = pfr.tile([P, 512], BF16, tag="hr")
                            nc.scalar.activation(out=hr[:], in_=ph[:],
                                                 func=AF.Relu)
                            nc.vector.tensor_mul(out=ht[:, ft, :], in0=hr[:],
                                                 in1=hr[:])
                        ysb = pfy.tile([P, 4, D], BF16, tag="ysb")
                        for tt in range(4):
                            py0 = psy.tile([P, 512], F32, space="PSUM",
                                           tag="py0")
                            py1 = psy.tile([P, 512], F32, space="PSUM",
                                           tag="py1")
                            for fc in range(FC):
                                nc.tensor.matmul(
                                    out=py0[:],
                                    lhsT=ht[:, fc, tt * P:(tt + 1) * P],
                                    rhs=wv_sb[:, fc, 0:512],
                                    start=(fc == 0), stop=(fc == FC - 1))
                                nc.tensor.matmul(
                                    out=py1[:],
                                    lhsT=ht[:, fc, tt * P:(tt + 1) * P],
                                    rhs=wv_sb[:, fc, 512:1024],
                                    start=(fc == 0), stop=(fc == FC - 1))
                            nc.scalar.activation(out=ysb[:, tt, 0:512],
                                                 in_=py0[:], func=AF.Copy)
                            nc.scalar.activation(out=ysb[:, tt, 512:1024],
                                                 in_=py1[:], func=AF.Copy)
                        cc = (elp, ck)
                        scol = (elp * NCK + ck) * 32
                        nc.gpsimd.dma_scatter_add(
                            out_ap=a2[cc][:], in_ap=ysb[:],
                            idxs_ap=sC[:, scol:scol + 32],
                            num_idxs=CH, num_idxs_reg=CH, elem_size=D)
                        nc.gpsimd.collective_compute(
                            "AllToAll", mybir.AluOpType.bypass,
                            replica_groups=rg,
                            ins=[a2[cc][0:NCORES * K2d[cc], :]],
                            outs=[recv2[OFF2[cc]:OFF2[cc] + NCORES * K2d[cc],
                                        :]])

            # ---------------- phase D: gather own rows, multiply by r
            with (
                tc.tile_pool(name="pdy", bufs=4) as pdy,
                tc.tile_pool(name="pdr", bufs=4) as pdr,
                tc.tile_pool(name="pd", bufs=2) as pd,
            ):
                rws = []
                for ck in range(T // 512):
                    rw = pdr.tile([P, 4, D], BF16, tag="rw")
                    nc.sync.dma_start(
                        out=rw[:],
                        in_=r_buf[ck * 512:(ck + 1) * 512, :].rearrange(
                            "(a p) d -> p a d", p=P))
                    rws.append(rw)
                ygs = []
                for ck in range(T // 512):
                    yg = pdy.tile([P, 4, D], BF16, tag="yg")
                    nc.gpsimd.dma_gather(
                        out_ap=yg[:], in_ap=recv2[:],
                        idxs_ap=yg16[:, ck * 32:(ck + 1) * 32],
                        num_idxs=512, num_idxs_reg=512, elem_size=D,
                        transpose=False)
                    ygs.append(yg)
                for ck in range(T // 512):
                    yo = pd.tile([P, 4, D], F32, tag="yo")
                    nc.vector.tensor_mul(out=yo[:], in0=ygs[ck][:],
                                         in1=rws[ck][:])
                    nc.scalar.dma_start(
                        out=out[ck * 512:(ck + 1) * 512, :].rearrange(
                            "(a p) d -> p a d", p=P),
                        in_=yo[:])

    nc.finalize()
    return nc


def _shuffle_rows(w, nchunks):
    """[R, ...] -> row p*nchunks+c holds original row c*128+p."""
    r = w.shape[0]
    assert r == nchunks * P
    return np.ascontiguousarray(
        w.reshape(nchunks, P, -1).transpose(1, 0, 2).reshape(w.shape))


def _prepare_inputs(x, token_ids, shift_state, time_maa_k, time_maa_r,
                    w_recept, w_key, w_value):
    cfg, idxs = _build_indices(token_ids)
    x = np.asarray(x, np.float32)
    shift = np.asarray(shift_state, np.float32)
    wrt = _shuffle_rows(
        np.ascontiguousarray(np.asarray(w_recept, np.float32).T), D // P
    ).astype(nbf16)
    wkb = np.asarray(w_key, np.float32).astype(nbf16)
    wkb = np.stack([_shuffle_rows(wkb[e], D // P) for e in range(E)])
    wvb = np.asarray(w_value, np.float32).astype(nbf16)
    wvb = np.stack([_shuffle_rows(wvb[e], F // P) for e in range(E)])
    mk = np.asarray(time_maa_k, np.float32)[None, :].astype(nbf16)
    mr = np.asarray(time_maa_r, np.float32)[None, :].astype(nbf16)
    # SBUF-source gather idx: output position j (= token q*512+j) reads
    # rank j//4 (partition), row j%4 -> idx value = (j%4)*128 + j//4
    j = np.arange(QT, dtype=np.int16)
    iota = _wrap16((j % 4) * P + j // 4)

    in_maps = []
    for k in range(NCORES):
        x_ext = np.concatenate([shift[k:k + 1], x[k]], axis=0).astype(nbf16)
        in_maps.append({
            "x_ext": np.ascontiguousarray(x_ext),
            "maa_k": mk, "maa_r": mr, "wrt": wrt,
            "wk": np.ascontiguousarray(wkb[EPC * k:EPC * (k + 1)]),
            "wv": np.ascontiguousarray(wvb[EPC * k:EPC * (k + 1)]),
            "iota16": iota,
            **idxs[k],
        })
    return cfg, in_maps


def kernel(x, token_ids, shift_state, time_maa_k, time_maa_r,
           w_recept, w_key, w_value, _trace=False):
    cfg, in_maps = _prepare_inputs(x, token_ids, shift_state, time_maa_k,
                                   time_maa_r, w_recept, w_key, w_value)
    if cfg not in _CACHE:
        _CACHE[cfg] = _build_nc(cfg)
    nc = _CACHE[cfg]
    res = run_bass_kernel_spmd(nc, in_maps, core_ids=list(range(NCORES)),
                               trace=_trace)
    kernel.last_result = res
    y = np.stack([res.results[k]["out"] for k in range(NCORES)], axis=0)
    return y.astype(np.float32)



# revision 9
# speedup vs baseline: 1.1136x; 1.1136x over previous
"""Expert-parallel CMoE kernel for 8 Trainium2 NeuronCores (v6a).

Sharding (hardcoded for B=8, T=2048, D=1024, F=2048, E=16, C=1024):
  core k owns batch k (token shift, receptance, output) and experts
  {2k, 2k+1} (FFN). Hash routing is int math on token_ids, done on host;
  the resulting permutations ship to the cores as index tensors.

v6a scheduling (driven by the v5 trace):
  - The scalar queue runs activations plus a few early/late DMAs; the
    v5 zero-fill clog (135us serialized, stalling sigmoids -> PSUM ->
    receptance -> dispatch scatters -> first A2A @280us) is gone.
  - Zero-fills use partition-contiguous "(p a) d" APs (4KB+ descriptors)
    on the gpsimd/scalar queues at harmless points.
  - Dispatch scatters run per 512-token chunk so the last scatter is
    small and the per-parity A2As trigger right at phase-A end.
  - A tiny warm-up AllToAll at t=0 absorbs the collective cold start
    (v5 paid 86us wire on the first A2A vs 25us warm).
  - recv1 is split per expert-parity so each XT gather depends only on
    its own dispatch A2A.
  - Expert-0 weights load during phase A on the scalar queue; expert-1
    reuses the same SBUF (tag-aliased) during expert-0 FFN2.
  - psr/prs are double-buffered so sigmoid evacuation never stalls the
    receptance matmuls.
Phase D (combine drain) is kept in the v5 shape: 4 A2As into one recv2,
then chunk gathers * r -> fp32 stores.
"""
import sys

for _p in ("/opt/trn_rl_repo", "/root/.axon_site/_ro/trn_rl_repo"):
    if _p not in sys.path:
        sys.path.append(_p)

import numpy as np
import ml_dtypes

import concourse.bass as bass
import concourse.bacc as bacc
import concourse.mybir as mybir
import concourse.tile as tile
from concourse.bass_utils import run_bass_kernel_spmd

P = 128
B, T, D, F, E = 8, 2048, 1024, 2048, 16
N = B * T
C = max(4, N // E)          # 1024
HASH_PRIME = 5099
NCORES = 8
EPC = E // NCORES           # experts per core = 2
NQ = 4                      # phase-A chunks
QT = T // NQ                # 512 tokens per chunk
NCK = 2                     # combine halves per expert
CH = C // NCK               # 512 slots per combine chunk
BF16 = mybir.dt.bfloat16
F32 = mybir.dt.float32
I16 = mybir.dt.int16
AF = mybir.ActivationFunctionType
nbf16 = ml_dtypes.bfloat16
ORDER = [(0, 0), (0, 1), (1, 0), (1, 1)]   # (parity, half) buckets

_CACHE = {}


def _r16(v):
    return int(-(-int(v) // 16) * 16)


def _wrap16(a):
    a = np.asarray(a, np.int16)
    w = a.reshape(-1, 16).T.copy()       # j at [j%16, j//16]
    return np.tile(w, (8, 1))            # replicated across 8 Q7 cores


def _route(token_ids):
    tid = np.asarray(token_ids).reshape(N).astype(np.int64)
    e = (tid * HASH_PRIME) % E
    onehot = (e[:, None] == np.arange(E)).astype(np.int64)
    pos = onehot.cumsum(0)[np.arange(N), e] - 1
    keep = pos < C
    return e, pos, keep


def _build_indices(token_ids):
    e, pos, keep = _route(token_ids)
    src = np.arange(N) // T
    dst = e // EPC
    el = e % EPC

    def pack(mask):
        rank = np.zeros(N, np.int64)
        cnt = np.zeros((NCORES, NCORES), np.int64)
        for n in np.nonzero(mask)[0]:
            rank[n] = cnt[src[n], dst[n]]
            cnt[src[n], dst[n]] += 1
        return rank, _r16(max(cnt.max(), 1))

    # ---- dispatch: one buffer per destination-expert parity
    de = [pack(keep & (el == p)) for p in range(EPC)]
    Kq = tuple(k for _, k in de)

    # recv1[e] row for slot pos of parity e on dst core: src*Kq+rank,
    # empty slots -> trash row NCORES*Kq[e]
    recv_row = [np.full((NCORES, C), NCORES * Kq[p], np.int64)
                for p in range(EPC)]
    for p in range(EPC):
        rank, K = de[p]
        for n in np.nonzero(keep & (el == p))[0]:
            recv_row[p][dst[n], pos[n]] = src[n] * K + rank[n]

    # ---- combine: 4 buckets (parity, capacity-half), single recv2
    comb = {c: pack(keep & (el == c[0]) & (pos // CH == c[1]))
            for c in ORDER}
    K2 = tuple(comb[c][1] for c in ORDER)
    K2d = dict(zip(ORDER, K2))
    OFF2 = {}
    acc = 0
    for c, k in zip(ORDER, K2):
        OFF2[c] = acc
        acc += NCORES * k
    R2 = acc                             # trash row in recv2

    sl2 = np.zeros((NCORES, EPC, C), np.int64)
    for c, k in zip(ORDER, K2):
        sl2[:, c[0], c[1] * CH:(c[1] + 1) * CH] = NCORES * k
    ygather = np.full(N, R2, np.int64)
    for n in np.nonzero(keep)[0]:
        c = (el[n], pos[n] // CH)
        rank, k = comb[c]
        sl2[dst[n], el[n], pos[n]] = src[n] * k + rank[n]
        ygather[n] = OFF2[c] + dst[n] * k + rank[n]

    per_core = []
    for k in range(NCORES):
        tok = slice(k * T, (k + 1) * T)
        # dispatch scatter idx per (chunk q, parity e): position j of the
        # chunk tile [P,4,D] <-> token q*512 + 4*(j%128) + j//128
        jj = np.arange(QT)
        tok_of_j = 4 * (jj % P) + jj // P
        sds = []
        for q in range(NQ):
            for p in range(EPC):
                rank, K = de[p]
                idx = np.where(keep & (el == p), dst * K + rank,
                               NCORES * K)[tok]
                sds.append(_wrap16(idx[q * QT + tok_of_j]))
        sd = np.concatenate(sds, axis=1)
        # slot gather idx (XT build): per parity, slots of expert 2k+p
        sl = np.concatenate(
            [_wrap16(recv_row[p][k]) for p in range(EPC)], axis=1)
        # combine scatter idx: position j = tt*128+p <-> slot c[1]*512+j
        sc = np.concatenate(
            [_wrap16(sl2[k, c[0], c[1] * CH:(c[1] + 1) * CH])
             for c in ORDER], axis=1)
        per_core.append({
            "sd16": sd, "slot16": sl, "sc16": sc,
            "ygather16": _wrap16(ygather[tok]),
        })
    return (Kq, K2), per_core


def _build_nc(cfg):
    Kq, K2 = cfg
    K2d = dict(zip(ORDER, K2))
    R1 = [NCORES * Kq[p] for p in range(EPC)]          # trash rows
    RB = [NCORES * K2d[c] for c in ORDER]              # bucket rows
    OFF2 = {}
    acc = 0
    for c, r in zip(ORDER, RB):
        OFF2[c] = acc
        acc += r
    R2 = acc

    nc = bacc.Bacc("TRN2", target_bir_lowering=False, debug=False,
                   num_devices=NCORES)

    x_ext = nc.dram_tensor("x_ext", [T + 1, D], BF16, kind="ExternalInput")
    maa_k = nc.dram_tensor("maa_k", [1, D], BF16, kind="ExternalInput")
    maa_r = nc.dram_tensor("maa_r", [1, D], BF16, kind="ExternalInput")
    wrt = nc.dram_tensor("wrt", [D, D], BF16, kind="ExternalInput")
    wk = nc.dram_tensor("wk", [EPC, D, F], BF16, kind="ExternalInput")
    wv = nc.dram_tensor("wv", [EPC, F, D], BF16, kind="ExternalInput")
    sd16 = nc.dram_tensor("sd16", [P, NQ * EPC * QT // 16], I16,
                          kind="ExternalInput")
    slot16 = nc.dram_tensor("slot16", [P, EPC * C // 16], I16,
                            kind="ExternalInput")
    sc16 = nc.dram_tensor("sc16", [P, EPC * C // 16], I16,
                          kind="ExternalInput")
    ygather16 = nc.dram_tensor("ygather16", [P, T // 16], I16,
                               kind="ExternalInput")
    iota16 = nc.dram_tensor("iota16", [P, QT // 16], I16,
                            kind="ExternalInput")
    out = nc.dram_tensor("out", [T, D], F32, kind="ExternalOutput")

    DC = D // P          # 8
    FC = F // P          # 16
    rg = [list(range(NCORES))]

    with tile.TileContext(nc) as tc:
        with (
            tc.tile_pool(name="dram", bufs=1, space="DRAM") as dram,
            tc.tile_pool(name="misc", bufs=1) as misc,
            tc.tile_pool(name="pwk", bufs=1) as pwk,
            tc.tile_pool(name="pwv", bufs=1) as pwv,
        ):
            disp = [dram.tile([R1[p] + 1, D], BF16, name=f"disp{p}")
                    for p in range(EPC)]
            recv1 = [dram.tile([R1[p] + 1, D], BF16, name=f"recv1_{p}")
                     for p in range(EPC)]
            a2 = {c: dram.tile([RB[i] + 1, D], BF16,
                               name=f"a2_{c[0]}_{c[1]}")
                  for i, c in enumerate(ORDER)}
            recv2 = dram.tile([R2 + 1, D], BF16)
            r_buf = dram.tile([T, D], BF16)
            warm_i = dram.tile([P, 64], BF16, name="warm_i")
            warm_o = dram.tile([P, 64], BF16, name="warm_o")

            # ---- warm-up A2A: absorbs the collective cold start
            nc.gpsimd.collective_compute(
                "AllToAll", mybir.AluOpType.bypass, replica_groups=rg,
                ins=[warm_i[:]], outs=[warm_o[:]])

            zbB = misc.tile([P, 2, D], BF16)       # bf16 zeros
            nc.vector.memzero(zbB[:])

            def zero_fill(q_eng, buf, rows):
                # partition-contiguous zero fill: one descriptor covers
                # 2 rows (4KB) per partition
                step = 2 * P
                for off in range(0, rows, step):
                    n = min(step, rows - off)
                    q_eng.dma_start(
                        out=buf[off:off + n, :].rearrange(
                            "(p a) d -> p a d", p=P),
                        in_=zbB[:, 0:n // P, :])

            # dispatch-buffer zero fill on gpsimd (ahead of the chunk
            # scatters in queue order)
            for p in range(EPC):
                zero_fill(nc.gpsimd, disp[p], R1[p])

            # trash rows (gathered by empty slots / dropped tokens)
            for p in range(EPC):
                nc.scalar.dma_start(out=recv1[p][R1[p]:R1[p] + 1, :],
                                    in_=zbB[0:1, 0, :])
            nc.scalar.dma_start(out=recv2[R2:R2 + 1, :],
                                in_=zbB[0:1, 0, :])

            # index loads on sync, ahead of the x chunk loads
            sD = misc.tile([P, NQ * EPC * QT // 16], I16)
            nc.sync.dma_start(out=sD[:], in_=sd16[:])
            sl16 = misc.tile([P, EPC * C // 16], I16)
            nc.sync.dma_start(out=sl16[:], in_=slot16[:])
            sC = misc.tile([P, EPC * C // 16], I16)
            nc.sync.dma_start(out=sC[:], in_=sc16[:])
            yg16 = misc.tile([P, T // 16], I16)
            nc.sync.dma_start(out=yg16[:], in_=ygather16[:])
            io16 = misc.tile([P, QT // 16], I16)
            nc.sync.dma_start(out=io16[:], in_=iota16[:])

            wk_t = [pwk.tile([P, DC, F], BF16, tag="wk", name=f"wk_t{i}")
                    for i in range(EPC)]
            wv_t = [pwv.tile([P, FC, D], BF16, tag="wv", name=f"wv_t{i}")
                    for i in range(EPC)]

            # expert-0 weights on the scalar queue (idle early; sigmoids
            # tolerate the ~25us head start)
            nc.scalar.dma_start(
                out=wk_t[0][:], in_=wk[0].rearrange("(p c) f -> p c f", p=P))
            nc.scalar.dma_start(
                out=wv_t[0][:], in_=wv[0].rearrange("(p c) f -> p c f", p=P))

            # ---- phase A: token shift + dispatch scatters + receptance
            with (
                tc.tile_pool(name="pa", bufs=2) as pa,
                tc.tile_pool(name="pxk", bufs=2) as pxk,
                tc.tile_pool(name="pdx", bufs=2) as pdx,
                tc.tile_pool(name="pam", bufs=1) as pam,
                tc.tile_pool(name="prx", bufs=2) as prx,
                tc.tile_pool(name="prs", bufs=2) as prs,
                tc.tile_pool(name="psr", bufs=2, space="PSUM") as psr,
            ):
                maakb = pam.tile([P, D], BF16)
                maarb = pam.tile([P, D], BF16)
                nc.sync.dma_start(out=maakb[:],
                                  in_=maa_k[:].to_broadcast([P, D]))
                nc.sync.dma_start(out=maarb[:],
                                  in_=maa_r[:].to_broadcast([P, D]))
                # wrt ships pre-shuffled so this is partition-contiguous
                wrt_sb = pam.tile([P, DC, D], BF16)
                nc.sync.dma_start(out=wrt_sb[:],
                                  in_=wrt.rearrange("(p c) e -> p c e", p=P))

                for q in range(NQ):
                    xq = pa.tile([P, 4, D], BF16, tag="xq")
                    nc.sync.dma_start(
                        out=xq[:],
                        in_=x_ext[1 + q * QT:1 + (q + 1) * QT, :].rearrange(
                            "(p a) d -> p a d", p=P))
                    # xprev strip for a=0: tokens 4p-1 = x_ext rows q*QT+4p
                    xp0 = pa.tile([P, 1, D], BF16, tag="xp0")
                    nc.sync.dma_start(
                        out=xp0[:],
                        in_=x_ext[q * QT:(q + 1) * QT, :].rearrange(
                            "(p a) d -> p a d", p=P)[:, 0:1, :])
                    dx = pdx.tile([P, 4, D], BF16, tag="dx")
                    nc.vector.tensor_sub(out=dx[:, 0:1, :], in0=xp0[:],
                                         in1=xq[:, 0:1, :])
                    nc.vector.tensor_sub(out=dx[:, 1:4, :],
                                         in0=xq[:, 0:3, :], in1=xq[:, 1:4, :])
                    xkq = pxk.tile([P, 4, D], BF16, tag="xkq")
                    for m in range(4):
                        nc.vector.tensor_mul(out=xkq[:, m, :],
                                             in0=dx[:, m, :], in1=maakb[:])
                    nc.vector.tensor_add(out=xkq[:], in0=xkq[:], in1=xq[:])

                    # dispatch this chunk (both parities) right away
                    for p in range(EPC):
                        nc.gpsimd.dma_scatter_add(
                            out_ap=disp[p][:], in_ap=xkq[:],
                            idxs_ap=sD[:, (q * EPC + p) * (QT // 16):
                                       (q * EPC + p + 1) * (QT // 16)],
                            num_idxs=QT, num_idxs_reg=QT, elem_size=D)

                    # xr built in-place in dx (dx is dead after this)
                    for m in range(4):
                        nc.vector.tensor_mul(out=dx[:, m, :],
                                             in0=dx[:, m, :], in1=maarb[:])
                    nc.vector.tensor_add(out=dx[:], in0=dx[:], in1=xq[:])

                    # receptance: SBUF-source transposing gather from dx
                    xrT = prx.tile([P, DC, QT], BF16, tag="xrT")
                    nc.gpsimd.dma_gather(
                        out_ap=xrT[:], in_ap=dx[:],
                        idxs_ap=io16[:],
                        num_idxs=QT, num_idxs_reg=QT, elem_size=D,
                        transpose=True,
                        sbuf_tokens_per_rank=P,
                        sbuf_free_dim_per_rank=D * 2)
                    if q == NQ - 1:
                        for p in range(EPC):
                            nc.gpsimd.collective_compute(
                                "AllToAll", mybir.AluOpType.bypass,
                                replica_groups=rg,
                                ins=[disp[p][0:R1[p], :]],
                                outs=[recv1[p][0:R1[p], :]])
                    rsb = prs.tile([P, 4, D], BF16, tag="rsb")
                    for tt in range(4):
                        pr0 = psr.tile([P, 512], F32, space="PSUM", tag="pr0")
                        pr1 = psr.tile([P, 512], F32, space="PSUM", tag="pr1")
                        for dc in range(DC):
                            nc.tensor.matmul(
                                out=pr0[:],
                                lhsT=xrT[:, dc, tt * P:(tt + 1) * P],
                                rhs=wrt_sb[:, dc, 0:512],
                                start=(dc == 0), stop=(dc == DC - 1))
                            nc.tensor.matmul(
                                out=pr1[:],
                                lhsT=xrT[:, dc, tt * P:(tt + 1) * P],
                                rhs=wrt_sb[:, dc, 512:1024],
                                start=(dc == 0), stop=(dc == DC - 1))
                        nc.scalar.activation(out=rsb[:, tt, 0:512],
                                             in_=pr0[:], func=AF.Sigmoid)
                        nc.scalar.activation(out=rsb[:, tt, 512:1024],
                                             in_=pr1[:], func=AF.Sigmoid)
                    nc.sync.dma_start(
                        out=r_buf[q * QT:(q + 1) * QT, :].rearrange(
                            "(a p) d -> p a d", p=P),
                        in_=rsb[:])

            # a2 zero-fills on the scalar queue after the phase-A
            # sigmoids; first needed mid-FFN of expert 0
            for i, c in enumerate(ORDER):
                zero_fill(nc.scalar, a2[c], RB[i])

            # ---------------- phase C: expert FFNs
            with (
                tc.tile_pool(name="pfx", bufs=2) as pfx,
                tc.tile_pool(name="pfh", bufs=1) as pfh,
                tc.tile_pool(name="pfr", bufs=2) as pfr,
                tc.tile_pool(name="pfy", bufs=2) as pfy,
                tc.tile_pool(name="psh", bufs=2, space="PSUM") as psh,
                tc.tile_pool(name="psy", bufs=2, space="PSUM") as psy,
            ):
                for elp in range(EPC):
                    if elp > 0:
                        nc.sync.dma_start(
                            out=wk_t[elp][:],
                            in_=wk[elp].rearrange("(p c) f -> p c f", p=P))
                        nc.sync.dma_start(
                            out=wv_t[elp][:],
                            in_=wv[elp].rearrange("(p c) f -> p c f", p=P))
                    wk_sb, wv_sb = wk_t[elp], wv_t[elp]
                    for ck in range(NCK):
                        i = elp * NCK + ck
                        XT = pfx.tile([P, DC, 512], BF16, tag="XT")
                        col0 = (elp * C + ck * CH) // 16
                        nc.gpsimd.dma_gather(
                            out_ap=XT[:], in_ap=recv1[elp][:],
                            idxs_ap=sl16[:, col0:col0 + 32],
                            num_idxs=512, num_idxs_reg=512, elem_size=D,
                            transpose=True)
                        ht = pfh.tile([P, FC, 512], BF16, tag="ht")
                        for ft in range(FC):
                            ph = psh.tile([P, 512], F32, space="PSUM",
                                          tag="ph")
                            for dc in range(DC):
                                nc.tensor.matmul(
                                    out=ph[:],
                                    lhsT=wk_sb[:, dc, ft * P:(ft + 1) * P],
                                    rhs=XT[:, dc, :],
                                    start=(dc == 0), stop=(dc == DC - 1))
                            hr = pfr.tile([P, 512], BF16, tag="hr")
                            nc.scalar.activation(out=hr[:], in_=ph[:],
                                                 func=AF.Relu)
                            nc.vector.tensor_mul(out=ht[:, ft, :], in0=hr[:],
                                                 in1=hr[:])
                        ysb = pfy.tile([P, 4, D], BF16, tag="ysb")
                        for tt in range(4):
                            py0 = psy.tile([P, 512], F32, space="PSUM",
                                           tag="py0")
                            py1 = psy.tile([P, 512], F32, space="PSUM",
                                           tag="py1")
                            for fc in range(FC):
                                nc.tensor.matmul(
                                    out=py0[:],
                                    lhsT=ht[:, fc, tt * P:(tt + 1) * P],
                                    rhs=wv_sb[:, fc, 0:512],
                                    start=(fc == 0), stop=(fc == FC - 1))
                                nc.tensor.matmul(
                                    out=py1[:],
                                    lhsT=ht[:, fc, tt * P:(tt + 1) * P],
                                    rhs=wv_sb[:, fc, 512:1024],
                                    start=(fc == 0), stop=(fc == FC - 1))
                            nc.scalar.activation(out=ysb[:, tt, 0:512],
                                                 in_=py0[:], func=AF.Copy)
                            nc.scalar.activation(out=ysb[:, tt, 512:1024],
                                                 in_=py1[:], func=AF.Copy)
                        cc = ORDER[i]
                        scol = i * 32
                        nc.gpsimd.dma_scatter_add(
                            out_ap=a2[cc][:], in_ap=ysb[:],
                            idxs_ap=sC[:, scol:scol + 32],
                            num_idxs=CH, num_idxs_reg=CH, elem_size=D)
                        nc.gpsimd.collective_compute(
                            "AllToAll", mybir.AluOpType.bypass,
                            replica_groups=rg,
                            ins=[a2[cc][0:RB[i], :]],
                            outs=[recv2[OFF2[cc]:OFF2[cc] + RB[i], :]])

            # ---------------- phase D: gather own rows, multiply by r
            with (
                tc.tile_pool(name="pdy", bufs=4) as pdy,
                tc.tile_pool(name="pdr", bufs=4) as pdr,
                tc.tile_pool(name="pd", bufs=2) as pd,
            ):
                rws = []
                for ck in range(T // 512):
                    rw = pdr.tile([P, 4, D], BF16, tag="rw")
                    nc.sync.dma_start(
                        out=rw[:],
                        in_=r_buf[ck * 512:(ck + 1) * 512, :].rearrange(
                            "(a p) d -> p a d", p=P))
                    rws.append(rw)
                ygs = []
                for ck in range(T // 512):
                    yg = pdy.tile([P, 4, D], BF16, tag="yg")
                    nc.gpsimd.dma_gather(
                        out_ap=yg[:], in_ap=recv2[:],
                        idxs_ap=yg16[:, ck * 32:(ck + 1) * 32],
                        num_idxs=512, num_idxs_reg=512, elem_size=D,
                        transpose=False)
                    ygs.append(yg)
                for ck in range(T // 512):
                    yo = pd.tile([P, 4, D], F32, tag="yo")
                    nc.vector.tensor_mul(out=yo[:], in0=ygs[ck][:],
                                         in1=rws[ck][:])
                    nc.scalar.dma_start(
                        out=out[ck * 512:(ck + 1) * 512, :].rearrange(
                            "(a p) d -> p a d", p=P),
                        in_=yo[:])

    nc.finalize()
    return nc


def _shuffle_rows(w, nchunks):
    """[R, ...] -> row p*nchunks+c holds original row c*128+p."""
    r = w.shape[0]
    assert r == nchunks * P
    return np.ascontiguousarray(
        w.reshape(nchunks, P, -1).transpose(1, 0, 2).reshape(w.shape))


def _prepare_inputs(x, token_ids, shift_state, time_maa_k, time_maa_r,
                    w_recept, w_key, w_value):
    cfg, idxs = _build_indices(token_ids)
    x = np.asarray(x, np.float32)
    shift = np.asarray(shift_state, np.float32)
    wrt = _shuffle_rows(
        np.ascontiguousarray(np.asarray(w_recept, np.float32).T), D // P
    ).astype(nbf16)
    wkb = np.asarray(w_key, np.float32).astype(nbf16)
    wkb = np.stack([_shuffle_rows(wkb[e], D // P) for e in range(E)])
    wvb = np.asarray(w_value, np.float32).astype(nbf16)
    wvb = np.stack([_shuffle_rows(wvb[e], F // P) for e in range(E)])
    mk = np.asarray(time_maa_k, np.float32)[None, :].astype(nbf16)
    mr = np.asarray(time_maa_r, np.float32)[None, :].astype(nbf16)
    # SBUF-source gather idx: output position j (= token q*512+j) reads
    # rank j//4 (partition), row j%4 -> idx value = (j%4)*128 + j//4
    j = np.arange(QT, dtype=np.int16)
    iota = _wrap16((j % 4) * P + j // 4)

    in_maps = []
    for k in range(NCORES):
        x_ext = np.concatenate([shift[k:k + 1], x[k]], axis=0).astype(nbf16)
        in_maps.append({
            "x_ext": np.ascontiguousarray(x_ext),
            "maa_k": mk, "maa_r": mr, "wrt": wrt,
            "wk": np.ascontiguousarray(wkb[EPC * k:EPC * (k + 1)]),
            "wv": np.ascontiguousarray(wvb[EPC * k:EPC * (k + 1)]),
            "iota16": iota,
            **idxs[k],
        })
    return cfg, in_maps


def kernel(x, token_ids, shift_state, time_maa_k, time_maa_r,
           w_recept, w_key, w_value, _trace=False):
    cfg, in_maps = _prepare_inputs(x, token_ids, shift_state, time_maa_k,
                                   time_maa_r, w_recept, w_key, w_value)
    if cfg not in _CACHE:
        _CACHE[cfg] = _build_nc(cfg)
    nc = _CACHE[cfg]
    res = run_bass_kernel_spmd(nc, in_maps, core_ids=list(range(NCORES)),
                               trace=_trace)
    kernel.last_result = res
    y = np.stack([res.results[k]["out"] for k in range(NCORES)], axis=0)
    return y.astype(np.float32)


# revision 12
# speedup vs baseline: 1.3237x; 1.1887x over previous
"""Expert-parallel CMoE kernel for 8 Trainium2 NeuronCores (v7).

Sharding (hardcoded for B=8, T=2048, D=1024, F=2048, E=16, C=1024):
  core k owns batch k (token shift, receptance, output) and experts
  {2k, 2k+1} (FFN). Hash routing is int math on token_ids, done on host;
  the resulting permutations ship to the cores as index tensors.

v7: NO dma_scatter_add anywhere.  HW traces showed each 512-row CCE-add
scatter costs ~30us of DMA (RMW) and scatters into one tile serialize
on each other's completion, so v5/v6 spent 120-260us routing.  Instead
every permutation is a read-only DMAGather (fast) plus a contiguous
store:
  dispatch: xk chunks -> xk_buf (contiguous store) -> per-512-row
    gather in disp-row order -> "(p a)" store into disp[parity] -> A2A.
  combine: ysb -> ybuf (contiguous store) -> gather in a2-row order ->
    "(p a)" store into a2[bucket] -> A2A.
Copy-writes cover every buffer row, so all zero-fills die with the
scatters (only the two recv trash rows are zeroed).

Other scheduling (from the v5/v6 traces):
  - scalar queue: activations + light stores only; index/x loads and
    xk stores on sync; disp/a2 gathers on gpsimd.
  - warm-up AllToAll at t=0 (first collective otherwise pays ~86us).
  - recv1 split per parity so each XT gather waits only its own A2A.
  - expert-0 weights load during phase A; expert-1 tag-aliases the
    same SBUF during expert-0 FFN2.
  - psr/prs double-buffered so sigmoids never stall receptance PE.
"""
import sys

for _p in ("/opt/trn_rl_repo", "/root/.axon_site/_ro/trn_rl_repo"):
    if _p not in sys.path:
        sys.path.append(_p)

import numpy as np
import ml_dtypes

import concourse.bass as bass
import concourse.bacc as bacc
import concourse.mybir as mybir
import concourse.tile as tile
from concourse.bass_utils import run_bass_kernel_spmd

P = 128
B, T, D, F, E = 8, 2048, 1024, 2048, 16
N = B * T
C = max(4, N // E)          # 1024
HASH_PRIME = 5099
NCORES = 8
EPC = E // NCORES           # experts per core = 2
NQ = 4                      # phase-A chunks
QT = T // NQ                # 512 tokens per chunk
NCK = 2                     # combine halves per expert
CH = C // NCK               # 512 slots per combine chunk
BF16 = mybir.dt.bfloat16
F32 = mybir.dt.float32
I16 = mybir.dt.int16
AF = mybir.ActivationFunctionType
nbf16 = ml_dtypes.bfloat16
ORDER = [(0, 0), (0, 1), (1, 0), (1, 1)]   # (parity, half) buckets

_CACHE = {}


def _r16(v):
    return int(-(-int(v) // 16) * 16)


def _wrap16(a):
    a = np.asarray(a, np.int16)
    w = a.reshape(-1, 16).T.copy()       # j at [j%16, j//16]
    return np.tile(w, (8, 1))            # replicated across 8 Q7 cores


def _perm_pa(idx_rows):
    """Permute a row-indexed idx array so a gather followed by a
    "(p a) d" store writes rows in order.

    Gather output position rr lands at tile[rr%128, rr//128]; the
    "(p a) d" store writes tile[p, a] to row p*A + a (A = nrows//128).
    So gather idx position rr must hold idx_rows[(rr%128)*A + rr//128].
    """
    n = len(idx_rows)
    A = n // P
    rr = np.arange(n)
    return idx_rows[(rr % P) * A + rr // P]


def _route(token_ids):
    tid = np.asarray(token_ids).reshape(N).astype(np.int64)
    e = (tid * HASH_PRIME) % E
    onehot = (e[:, None] == np.arange(E)).astype(np.int64)
    pos = onehot.cumsum(0)[np.arange(N), e] - 1
    keep = pos < C
    return e, pos, keep


def _build_indices(token_ids):
    e, pos, keep = _route(token_ids)
    src = np.arange(N) // T
    dst = e // EPC
    el = e % EPC

    def pack(mask):
        rank = np.zeros(N, np.int64)
        cnt = np.zeros((NCORES, NCORES), np.int64)
        for n in np.nonzero(mask)[0]:
            rank[n] = cnt[src[n], dst[n]]
            cnt[src[n], dst[n]] += 1
        return rank, _r16(max(cnt.max(), 1))

    # ---- dispatch: one buffer per destination-expert parity
    de = [pack(keep & (el == p)) for p in range(EPC)]
    Kq = tuple(k for _, k in de)

    # recv1[e] row for slot pos of parity e on dst core: src*Kq+rank,
    # empty slots -> trash row NCORES*Kq[e]
    recv_row = [np.full((NCORES, C), NCORES * Kq[p], np.int64)
                for p in range(EPC)]
    # disp[e] row -> local token on the source core (pads -> 0)
    disp_tok = [np.zeros((NCORES, NCORES * Kq[p]), np.int64)
                for p in range(EPC)]
    for p in range(EPC):
        rank, K = de[p]
        for n in np.nonzero(keep & (el == p))[0]:
            recv_row[p][dst[n], pos[n]] = src[n] * K + rank[n]
            disp_tok[p][src[n], dst[n] * K + rank[n]] = n - src[n] * T

    # ---- combine: 4 buckets (parity, capacity-half), single recv2
    comb = {c: pack(keep & (el == c[0]) & (pos // CH == c[1]))
            for c in ORDER}
    K2 = tuple(comb[c][1] for c in ORDER)
    K2d = dict(zip(ORDER, K2))
    OFF2 = {}
    acc = 0
    for c, k in zip(ORDER, K2):
        OFF2[c] = acc
        acc += NCORES * k
    R2 = acc                             # trash row in recv2

    # a2[bucket] row -> local slot (within the bucket's 512) on the
    # expert core (pads -> 0)
    a2_slot = {c: np.zeros((NCORES, NCORES * K2d[c]), np.int64)
               for c in ORDER}
    ygather = np.full(N, R2, np.int64)
    for n in np.nonzero(keep)[0]:
        c = (el[n], pos[n] // CH)
        rank, k = comb[c]
        a2_slot[c][dst[n], src[n] * k + rank[n]] = pos[n] % CH
        ygather[n] = OFF2[c] + dst[n] * k + rank[n]

    per_core = []
    for k in range(NCORES):
        tok = slice(k * T, (k + 1) * T)
        # dispatch gather idx: disp rows in 512-row pieces, permuted for
        # the "(p a) d" store
        dgs = []
        for p in range(EPC):
            rows = disp_tok[p][k]
            for j in range(0, len(rows), QT):
                dgs.append(_wrap16(_perm_pa(rows[j:j + QT])))
        dg = np.concatenate(dgs, axis=1)
        # combine gather idx: a2 rows in 512-row pieces
        ags = []
        for c in ORDER:
            rows = a2_slot[c][k]
            for j in range(0, len(rows), QT):
                ags.append(_wrap16(_perm_pa(rows[j:j + QT])))
        ag = np.concatenate(ags, axis=1)
        # slot gather idx (XT build): per parity, slots of expert 2k+p
        sl = np.concatenate(
            [_wrap16(recv_row[p][k]) for p in range(EPC)], axis=1)
        per_core.append({
            "dg16": dg, "ag16": ag, "slot16": sl,
            "ygather16": _wrap16(ygather[tok]),
        })
    return (Kq, K2), per_core


def _build_nc(cfg):
    Kq, K2 = cfg
    K2d = dict(zip(ORDER, K2))
    R1 = [NCORES * Kq[p] for p in range(EPC)]          # trash rows
    RB = [NCORES * K2d[c] for c in ORDER]              # bucket rows
    OFF2 = {}
    acc = 0
    for c, r in zip(ORDER, RB):
        OFF2[c] = acc
        acc += r
    R2 = acc
    DGC = sum(R1) // 16                                # dg16 cols
    AGC = sum(RB) // 16                                # ag16 cols

    nc = bacc.Bacc("TRN2", target_bir_lowering=False, debug=False,
                   num_devices=NCORES)

    x_ext = nc.dram_tensor("x_ext", [T + 1, D], BF16, kind="ExternalInput")
    maa_k = nc.dram_tensor("maa_k", [1, D], BF16, kind="ExternalInput")
    maa_r = nc.dram_tensor("maa_r", [1, D], BF16, kind="ExternalInput")
    wrt = nc.dram_tensor("wrt", [D, D], BF16, kind="ExternalInput")
    wk = nc.dram_tensor("wk", [EPC, D, F], BF16, kind="ExternalInput")
    wv = nc.dram_tensor("wv", [EPC, F, D], BF16, kind="ExternalInput")
    dg16 = nc.dram_tensor("dg16", [P, DGC], I16, kind="ExternalInput")
    ag16 = nc.dram_tensor("ag16", [P, AGC], I16, kind="ExternalInput")
    slot16 = nc.dram_tensor("slot16", [P, EPC * C // 16], I16,
                            kind="ExternalInput")
    ygather16 = nc.dram_tensor("ygather16", [P, T // 16], I16,
                               kind="ExternalInput")
    iota16 = nc.dram_tensor("iota16", [P, QT // 16], I16,
                            kind="ExternalInput")
    out = nc.dram_tensor("out", [T, D], F32, kind="ExternalOutput")

    DC = D // P          # 8
    FC = F // P          # 16
    rg = [list(range(NCORES))]

    with tile.TileContext(nc) as tc:
        with (
            tc.tile_pool(name="dram", bufs=1, space="DRAM") as dram,
            tc.tile_pool(name="misc", bufs=1) as misc,
            tc.tile_pool(name="pwk", bufs=1) as pwk,
            tc.tile_pool(name="pwv", bufs=1) as pwv,
            tc.tile_pool(name="pdg", bufs=2) as pdg,
        ):
            xk_buf = dram.tile([T, D], BF16, name="xk_buf")
            disp = [dram.tile([R1[p], D], BF16, name=f"disp{p}")
                    for p in range(EPC)]
            recv1 = [dram.tile([R1[p] + 1, D], BF16, name=f"recv1_{p}")
                     for p in range(EPC)]
            ybuf = [dram.tile([CH, D], BF16, name=f"ybuf{i}")
                    for i in range(EPC * NCK)]
            a2 = {c: dram.tile([RB[i], D], BF16, name=f"a2_{c[0]}_{c[1]}")
                  for i, c in enumerate(ORDER)}
            recv2 = dram.tile([R2 + 1, D], BF16)
            r_buf = dram.tile([T, D], BF16)
            warm_i = dram.tile([P, 64], BF16, name="warm_i")
            warm_o = dram.tile([P, 64], BF16, name="warm_o")

            # ---- warm-up A2A: absorbs the collective cold start
            nc.gpsimd.collective_compute(
                "AllToAll", mybir.AluOpType.bypass, replica_groups=rg,
                ins=[warm_i[:]], outs=[warm_o[:]])

            zrow = misc.tile([1, D], BF16)
            nc.vector.memzero(zrow[:])
            # trash rows (gathered by empty slots / dropped tokens)
            for p in range(EPC):
                nc.scalar.dma_start(out=recv1[p][R1[p]:R1[p] + 1, :],
                                    in_=zrow[:])
            nc.scalar.dma_start(out=recv2[R2:R2 + 1, :], in_=zrow[:])

            # index loads on sync, ahead of the x chunk loads
            dG = misc.tile([P, DGC], I16)
            nc.sync.dma_start(out=dG[:], in_=dg16[:])
            aG = misc.tile([P, AGC], I16)
            nc.sync.dma_start(out=aG[:], in_=ag16[:])
            sl16 = misc.tile([P, EPC * C // 16], I16)
            nc.sync.dma_start(out=sl16[:], in_=slot16[:])
            yg16 = misc.tile([P, T // 16], I16)
            nc.sync.dma_start(out=yg16[:], in_=ygather16[:])
            io16 = misc.tile([P, QT // 16], I16)
            nc.sync.dma_start(out=io16[:], in_=iota16[:])

            wk_t = [pwk.tile([P, DC, F], BF16, tag="wk", name=f"wk_t{i}")
                    for i in range(EPC)]
            wv_t = [pwv.tile([P, FC, D], BF16, tag="wv", name=f"wv_t{i}")
                    for i in range(EPC)]

            # expert-0 weights on the scalar queue (idle early; sigmoids
            # tolerate the ~25us head start)
            nc.scalar.dma_start(
                out=wk_t[0][:], in_=wk[0].rearrange("(p c) f -> p c f", p=P))
            nc.scalar.dma_start(
                out=wv_t[0][:], in_=wv[0].rearrange("(p c) f -> p c f", p=P))

            def permute_rows(src_buf, dst_buf, rows, idx_sb, col0,
                             st_eng):
                """dst rows <- src rows per idx, in 512-row pieces:
                gather (gpsimd) + "(p a) d" store (st_eng)."""
                for j in range(0, rows, QT):
                    n = min(QT, rows - j)
                    bt = pdg.tile([P, QT // P, D], BF16, tag="bt")
                    an = n // P
                    nc.gpsimd.dma_gather(
                        out_ap=bt[:, 0:an, :], in_ap=src_buf[:],
                        idxs_ap=idx_sb[:, col0:col0 + n // 16],
                        num_idxs=n, num_idxs_reg=n, elem_size=D,
                        transpose=False)
                    st_eng.dma_start(
                        out=dst_buf[j:j + n, :].rearrange(
                            "(p a) d -> p a d", p=P),
                        in_=bt[:, 0:an, :])
                    col0 += n // 16
                return col0

            # ---- phase A: token shift + receptance
            with (
                tc.tile_pool(name="pa", bufs=2) as pa,
                tc.tile_pool(name="pxk", bufs=2) as pxk,
                tc.tile_pool(name="pdx", bufs=2) as pdx,
                tc.tile_pool(name="pam", bufs=1) as pam,
                tc.tile_pool(name="prx", bufs=2) as prx,
                tc.tile_pool(name="prs", bufs=2) as prs,
                tc.tile_pool(name="psr", bufs=2, space="PSUM") as psr,
            ):
                maakb = pam.tile([P, D], BF16)
                maarb = pam.tile([P, D], BF16)
                nc.sync.dma_start(out=maakb[:],
                                  in_=maa_k[:].to_broadcast([P, D]))
                nc.sync.dma_start(out=maarb[:],
                                  in_=maa_r[:].to_broadcast([P, D]))
                # wrt ships pre-shuffled so this is partition-contiguous
                wrt_sb = pam.tile([P, DC, D], BF16)
                nc.sync.dma_start(out=wrt_sb[:],
                                  in_=wrt.rearrange("(p c) e -> p c e", p=P))

                for q in range(NQ):
                    xq = pa.tile([P, 4, D], BF16, tag="xq")
                    nc.sync.dma_start(
                        out=xq[:],
                        in_=x_ext[1 + q * QT:1 + (q + 1) * QT, :].rearrange(
                            "(p a) d -> p a d", p=P))
                    # xprev strip for a=0: tokens 4p-1 = x_ext rows q*QT+4p
                    xp0 = pa.tile([P, 1, D], BF16, tag="xp0")
                    nc.sync.dma_start(
                        out=xp0[:],
                        in_=x_ext[q * QT:(q + 1) * QT, :].rearrange(
                            "(p a) d -> p a d", p=P)[:, 0:1, :])
                    dx = pdx.tile([P, 4, D], BF16, tag="dx")
                    nc.vector.tensor_sub(out=dx[:, 0:1, :], in0=xp0[:],
                                         in1=xq[:, 0:1, :])
                    nc.vector.tensor_sub(out=dx[:, 1:4, :],
                                         in0=xq[:, 0:3, :], in1=xq[:, 1:4, :])
                    xkq = pxk.tile([P, 4, D], BF16, tag="xkq")
                    for m in range(4):
                        nc.vector.tensor_mul(out=xkq[:, m, :],
                                             in0=dx[:, m, :], in1=maakb[:])
                    nc.vector.tensor_add(out=xkq[:], in0=xkq[:], in1=xq[:])
                    # xk chunk to DRAM, contiguous: row 4p+a <- [p, a]
                    nc.sync.dma_start(
                        out=xk_buf[q * QT:(q + 1) * QT, :].rearrange(
                            "(p a) d -> p a d", p=P),
                        in_=xkq[:])

                    # xr built in-place in dx (dx is dead after this)
                    for m in range(4):
                        nc.vector.tensor_mul(out=dx[:, m, :],
                                             in0=dx[:, m, :], in1=maarb[:])
                    nc.vector.tensor_add(out=dx[:], in0=dx[:], in1=xq[:])

                    # receptance: SBUF-source transposing gather from dx
                    xrT = prx.tile([P, DC, QT], BF16, tag="xrT")
                    nc.gpsimd.dma_gather(
                        out_ap=xrT[:], in_ap=dx[:],
                        idxs_ap=io16[:],
                        num_idxs=QT, num_idxs_reg=QT, elem_size=D,
                        transpose=True,
                        sbuf_tokens_per_rank=P,
                        sbuf_free_dim_per_rank=D * 2)
                    if q == NQ - 1:
                        # dispatch: gather xk_buf rows into disp order,
                        # store, then A2A -- per parity
                        col = 0
                        for p in range(EPC):
                            col = permute_rows(xk_buf, disp[p], R1[p],
                                               dG, col, nc.sync)
                            nc.gpsimd.collective_compute(
                                "AllToAll", mybir.AluOpType.bypass,
                                replica_groups=rg,
                                ins=[disp[p][:]],
                                outs=[recv1[p][0:R1[p], :]])
                    rsb = prs.tile([P, 4, D], BF16, tag="rsb")
                    for tt in range(4):
                        pr0 = psr.tile([P, 512], F32, space="PSUM", tag="pr0")
                        pr1 = psr.tile([P, 512], F32, space="PSUM", tag="pr1")
                        for dc in range(DC):
                            nc.tensor.matmul(
                                out=pr0[:],
                                lhsT=xrT[:, dc, tt * P:(tt + 1) * P],
                                rhs=wrt_sb[:, dc, 0:512],
                                start=(dc == 0), stop=(dc == DC - 1))
                            nc.tensor.matmul(
                                out=pr1[:],
                                lhsT=xrT[:, dc, tt * P:(tt + 1) * P],
                                rhs=wrt_sb[:, dc, 512:1024],
                                start=(dc == 0), stop=(dc == DC - 1))
                        nc.scalar.activation(out=rsb[:, tt, 0:512],
                                             in_=pr0[:], func=AF.Sigmoid)
                        nc.scalar.activation(out=rsb[:, tt, 512:1024],
                                             in_=pr1[:], func=AF.Sigmoid)
                    nc.scalar.dma_start(
                        out=r_buf[q * QT:(q + 1) * QT, :].rearrange(
                            "(a p) d -> p a d", p=P),
                        in_=rsb[:])

            # ---------------- phase C: expert FFNs
            with (
                tc.tile_pool(name="pfx", bufs=2) as pfx,
                tc.tile_pool(name="pfh", bufs=1) as pfh,
                tc.tile_pool(name="pfr", bufs=2) as pfr,
                tc.tile_pool(name="pfy", bufs=2) as pfy,
                tc.tile_pool(name="psh", bufs=2, space="PSUM") as psh,
                tc.tile_pool(name="psy", bufs=2, space="PSUM") as psy,
            ):
                agcol = [0]
                for i, c in enumerate(ORDER):
                    agcol.append(agcol[-1] + RB[i] // 16)
                for elp in range(EPC):
                    if elp > 0:
                        nc.sync.dma_start(
                            out=wk_t[elp][:],
                            in_=wk[elp].rearrange("(p c) f -> p c f", p=P))
                        nc.sync.dma_start(
                            out=wv_t[elp][:],
                            in_=wv[elp].rearrange("(p c) f -> p c f", p=P))
                    wk_sb, wv_sb = wk_t[elp], wv_t[elp]
                    for ck in range(NCK):
                        i = elp * NCK + ck
                        XT = pfx.tile([P, DC, 512], BF16, tag="XT")
                        col0 = (elp * C + ck * CH) // 16
                        nc.gpsimd.dma_gather(
                            out_ap=XT[:], in_ap=recv1[elp][:],
                            idxs_ap=sl16[:, col0:col0 + 32],
                            num_idxs=512, num_idxs_reg=512, elem_size=D,
                            transpose=True)
                        ht = pfh.tile([P, FC, 512], BF16, tag="ht")
                        for ft in range(FC):
                            ph = psh.tile([P, 512], F32, space="PSUM",
                                          tag="ph")
                            for dc in range(DC):
                                nc.tensor.matmul(
                                    out=ph[:],
                                    lhsT=wk_sb[:, dc, ft * P:(ft + 1) * P],
                                    rhs=XT[:, dc, :],
                                    start=(dc == 0), stop=(dc == DC - 1))
                            hr = pfr.tile([P, 512], BF16, tag="hr")
                            nc.scalar.activation(out=hr[:], in_=ph[:],
                                                 func=AF.Relu)
                            nc.vector.tensor_mul(out=ht[:, ft, :], in0=hr[:],
                                                 in1=hr[:])
                        ysb = pfy.tile([P, 4, D], BF16, tag="ysb")
                        for tt in range(4):
                            py0 = psy.tile([P, 512], F32, space="PSUM",
                                           tag="py0")
                            py1 = psy.tile([P, 512], F32, space="PSUM",
                                           tag="py1")
                            for fc in range(FC):
                                nc.tensor.matmul(
                                    out=py0[:],
                                    lhsT=ht[:, fc, tt * P:(tt + 1) * P],
                                    rhs=wv_sb[:, fc, 0:512],
                                    start=(fc == 0), stop=(fc == FC - 1))
                                nc.tensor.matmul(
                                    out=py1[:],
                                    lhsT=ht[:, fc, tt * P:(tt + 1) * P],
                                    rhs=wv_sb[:, fc, 512:1024],
                                    start=(fc == 0), stop=(fc == FC - 1))
                            nc.scalar.activation(out=ysb[:, tt, 0:512],
                                                 in_=py0[:], func=AF.Copy)
                            nc.scalar.activation(out=ysb[:, tt, 512:1024],
                                                 in_=py1[:], func=AF.Copy)
                        cc = ORDER[i]
                        # slot-ordered store: ysb[p, tt] = slot tt*128+p
                        nc.sync.dma_start(
                            out=ybuf[i][:].rearrange("(a p) d -> p a d",
                                                     p=P),
                            in_=ysb[:])
                        # a2 rows <- ybuf rows per ag16, then A2A
                        permute_rows(ybuf[i], a2[cc], RB[i], aG,
                                     int(agcol[i]), nc.sync)
                        nc.gpsimd.collective_compute(
                            "AllToAll", mybir.AluOpType.bypass,
                            replica_groups=rg,
                            ins=[a2[cc][:]],
                            outs=[recv2[OFF2[cc]:OFF2[cc] + RB[i], :]])

            # ---------------- phase D: gather own rows, multiply by r
            with (
                tc.tile_pool(name="pdy", bufs=4) as pdy,
                tc.tile_pool(name="pdr", bufs=4) as pdr,
                tc.tile_pool(name="pd", bufs=2) as pd,
            ):
                rws = []
                for ck in range(T // 512):
                    rw = pdr.tile([P, 4, D], BF16, tag="rw")
                    nc.sync.dma_start(
                        out=rw[:],
                        in_=r_buf[ck * 512:(ck + 1) * 512, :].rearrange(
                            "(a p) d -> p a d", p=P))
                    rws.append(rw)
                ygs = []
                for ck in range(T // 512):
                    yg = pdy.tile([P, 4, D], BF16, tag="yg")
                    nc.gpsimd.dma_gather(
                        out_ap=yg[:], in_ap=recv2[:],
                        idxs_ap=yg16[:, ck * 32:(ck + 1) * 32],
                        num_idxs=512, num_idxs_reg=512, elem_size=D,
                        transpose=False)
                    ygs.append(yg)
                for ck in range(T // 512):
                    yo = pd.tile([P, 4, D], F32, tag="yo")
                    nc.vector.tensor_mul(out=yo[:], in0=ygs[ck][:],
                                         in1=rws[ck][:])
                    nc.scalar.dma_start(
                        out=out[ck * 512:(ck + 1) * 512, :].rearrange(
                            "(a p) d -> p a d", p=P),
                        in_=yo[:])

    nc.finalize()
    return nc


def _shuffle_rows(w, nchunks):
    """[R, ...] -> row p*nchunks+c holds original row c*128+p."""
    r = w.shape[0]
    assert r == nchunks * P
    return np.ascontiguousarray(
        w.reshape(nchunks, P, -1).transpose(1, 0, 2).reshape(w.shape))


def _prepare_inputs(x, token_ids, shift_state, time_maa_k, time_maa_r,
                    w_recept, w_key, w_value):
    cfg, idxs = _build_indices(token_ids)
    x = np.asarray(x, np.float32)
    shift = np.asarray(shift_state, np.float32)
    wrt = _shuffle_rows(
        np.ascontiguousarray(np.asarray(w_recept, np.float32).T), D // P
    ).astype(nbf16)
    wkb = np.asarray(w_key, np.float32).astype(nbf16)
    wkb = np.stack([_shuffle_rows(wkb[e], D // P) for e in range(E)])
    wvb = np.asarray(w_value, np.float32).astype(nbf16)
    wvb = np.stack([_shuffle_rows(wvb[e], F // P) for e in range(E)])
    mk = np.asarray(time_maa_k, np.float32)[None, :].astype(nbf16)
    mr = np.asarray(time_maa_r, np.float32)[None, :].astype(nbf16)
    # SBUF-source gather idx: output position j (= token q*512+j) reads
    # rank j//4 (partition), row j%4 -> idx value = (j%4)*128 + j//4
    j = np.arange(QT, dtype=np.int16)
    iota = _wrap16((j % 4) * P + j // 4)

    in_maps = []
    for k in range(NCORES):
        x_ext = np.concatenate([shift[k:k + 1], x[k]], axis=0).astype(nbf16)
        in_maps.append({
            "x_ext": np.ascontiguousarray(x_ext),
            "maa_k": mk, "maa_r": mr, "wrt": wrt,
            "wk": np.ascontiguousarray(wkb[EPC * k:EPC * (k + 1)]),
            "wv": np.ascontiguousarray(wvb[EPC * k:EPC * (k + 1)]),
            "iota16": iota,
            **idxs[k],
        })
    return cfg, in_maps


def kernel(x, token_ids, shift_state, time_maa_k, time_maa_r,
           w_recept, w_key, w_value, _trace=False):
    cfg, in_maps = _prepare_inputs(x, token_ids, shift_state, time_maa_k,
                                   time_maa_r, w_recept, w_key, w_value)
    if cfg not in _CACHE:
        _CACHE[cfg] = _build_nc(cfg)
    nc = _CACHE[cfg]
    res = run_bass_kernel_spmd(nc, in_maps, core_ids=list(range(NCORES)),
                               trace=_trace)
    kernel.last_result = res
    y = np.stack([res.results[k]["out"] for k in range(NCORES)], axis=0)
    return y.astype(np.float32)
